# revision 17
# baseline (speedup 1.0000x reference)
"""Trainium2 Bass kernel for ExpertMLP: out = relu(x @ W_fc.T)^2 @ W_proj.T.

Sharding: 4-way tokens x 2-way hidden across 8 NeuronCores.
Each core computes a partial out^T[:, t_shard] contracted over its hidden
half; the host sums the two hidden halves (fp16 partials, upcast to fp32)
and transposes while unsharding.

Per-core kernel (T_S=2048 tokens, HID_S=2048 hidden, DIM=1024), fp16
matmul operands with fp32 PSUM accumulation:
  mm1: h^T[j, t] = W_fcT-chunks.T @ xT-chunks     (PSUM accum over d)
  act: relu^2 (DVE max(ps,0) -> fp16, DVE square)
  mm2: out^T[d, t] = W_projT-chunks.T @ h^T-chunks (PSUM accum over j)

Measured design notes (tight reps-delta benches, 8-core SPMD):
- Sustained matmul rate is data-dependent (power throttling): all-zero
  fp16 operands stream at ~216 ns/MM (2.4 GHz), dense-random fp16 at
  ~310 ns/MM. (bf16 measured faster on a matmul-only probe but slower
  in the full kernel, so operands stay fp16.)
- Inputs are rounded host-side (fp16 kept): x to 5 mantissa bits,
  weights to 6. The PE multiplies at FP22, so zeroed low mantissa bits
  toggle fewer partial products, raising the throttled clock (~15us/body
  total). The moving operand's bits matter most (they stream through the
  array every cycle; stationary weight bits are static gates — weights
  at m=5 measured no further gain). Quantization cost: rel err 9.9e-3
  vs 6.8e-4 untruncated (gate 2e-2).
- PSUM accumulation groups are [P, 1024] 2-bank tiles on a 4-deep tag
  rotation, so a bank pair is reused 3 groups (~10us) after its drain.
- mm2 evictions run on ScalarE (Copy activation, fp32->fp16); mm1 drains
  on DVE. Splitting PSUM readers across engines measured fastest.
- For_i carries an all-engine barrier per iteration; unrolling 2 bodies
  per iteration amortizes it (~4us/body).
- Weights and x stay SBUF-resident; only x (in, once) and out^T (out,
  per body) touch HBM. Output is fp16 partials (halves the out DMA).

Power-wall characterization (same-session reps-delta A/Bs, 2026-08-10):
- Structural floor: the SAME NEFF with x zeroed runs 221.5us =
  1024 MM x 216 ns (the warm 2.4 GHz rate) -- zero idle/stall slack.
  All of the ~47 ns/MM real-data overhead is power throttling.
- Phase split: mm1-only 265 ns/MM, mm2-only 266 ns/MM -- both phases
  equally throttled even though mm2 streams full-mantissa fp16 act.
- The operand-mantissa knob is saturated at m=5/6: truncating inputs
  further (x to m=2 AND w to m=4, data-only change on the same NEFF)
  measures 269.6us vs base 268.9us. Streaming act as fp8 E3M4
  (mixed-dtype matmul, rel err 1.65e-2) measures 267.9us: noise.
  Halving LDWEIGHTS count (nth=4, 2048-token PSUM groups) is also
  noise (271.2us). The residual throttle is pinned by value-entropy
  energy in the accumulate datapath (partial-sum ripple + PSUM RMW),
  which no operand format reaches; only all-zero streams drop it.
- scalar_tensor_tensor relu^2 fusion from PSUM is illegal: walrus
  rejects two non-scalar PSUM reads per instruction (NCC_IBVF027).
- Session-to-session drift of the throttled rate is ~5% (268-282us
  for identical NEFFs+data); A/B only within one process.
- Closed in round 5: N=1024 fp16 moving operand is ISA-illegal
  (NCC_IXCG864; one PSUM bank caps N at 512 fp32 outputs, and the
  cost model confirms 1 col/cycle for all 16-bit dtypes -- the
  "131 ns/MM @ N=512" doc line does not apply). Removing the out-DMA
  entirely is worth <=0.5% (267.5 vs 268.1/269.2 same-session), so
  fp8 output partials are not worth the accuracy. fp8 DoubleRow is
  unreachable: its pair interleave needs two j-rows per SBUF
  partition, a cross-partition transpose DVE/ACT/DMA cannot do
  efficiently, on top of failing the 2e-2 gate (e4m3 act = 3.2e-2).
"""

import numpy as np

import concourse.mybir as mybir
import concourse.tile as tile
from concourse import bacc
from concourse import bass_utils

T, DIM, HID = 8192, 1024, 4096
N_CORES = 8
TOK_WAYS, HID_WAYS = 4, 2
T_S = T // TOK_WAYS        # 2048 tokens per core
HID_S = HID // HID_WAYS    # 2048 hidden units per core
P = 128
F32 = mybir.dt.float32
F16 = mybir.dt.float16

T_CHUNK = 512              # free dim per matmul (one PSUM bank)
NTH = 2                    # t-chunks per accumulation group
T_HALF = T_CHUNK * NTH     # 1024 tokens per group

KD = DIM // P              # 8 contraction chunks for mm1
JC = HID_S // P            # 16 j-chunks (also mm2 contraction chunks)
DC = DIM // P              # 8 output-dim chunks for mm2

UNROLL = 2                 # bodies per For_i iteration


def build_nc(reps: int = 1, relu_engine: str = "dve", staggered: bool = True,
             unroll: int | None = None, act_dt: str = "f16",
             act_mode: str = "maxmul", nth: int = NTH, parts: str = "both",
             mm_n: int = T_CHUNK, skip_dma: bool = False):
    """staggered=True defers the For_i semaphore-reset barrier past the
    per-iteration pipeline drain (measured ~1us/body vs plain in a
    same-window A/B; never worse on either estimator).

    act_dt: "f16" or "f8e3" — storage dtype of the relu^2 activations
      (mm2's moving operand). f8e3 (E3M4) halves the SBUF->PE streaming
      bus width for mm2; mixed-dtype matmul (fp8 moving x fp16
      stationary) is legal on trn2 (both are upconverted to FP22).
    act_mode: "maxmul" (DVE max + DVE mul, 2 ops) or "stt" (single DVE
      scalar_tensor_tensor computing (ps max 0) * ps = relu(ps)^2).
    nth: 512-token chunks per PSUM accumulation group (2 or 4). nth=4
      halves LDWEIGHTS count (each weight serves 2048 tokens) at the
      cost of a 2-deep (vs 4-deep) PSUM tag rotation.
    """
    ACT = F16 if act_dt == "f16" else mybir.dt.float8e3
    n_tags = 8 // nth          # PSUM: group = nth banks
    t_half = T_CHUNK * nth     # tokens per accumulation group
    th_groups = T_S // t_half
    assert mm_n % T_CHUNK == 0 and t_half % mm_n == 0
    tsub = mm_n // T_CHUNK     # 512-chunks per matmul instruction
    nc = bacc.Bacc("TRN2", target_bir_lowering=False, debug=False)
    xT = nc.dram_tensor("xT", [DIM, T_S], F16, kind="ExternalInput")
    wfcT = nc.dram_tensor("wfcT", [DIM, HID_S], F16, kind="ExternalInput")
    wprojT = nc.dram_tensor("wprojT", [HID_S, DIM], F16, kind="ExternalInput")
    outT = nc.dram_tensor("outT", [DIM, T_S], F16, kind="ExternalOutput")

    xT_r = xT.ap().rearrange("(o p) t -> p o t", p=P)
    wfcT_r = wfcT.ap().rearrange("(o p) h -> p o h", p=P)
    wprojT_r = wprojT.ap().rearrange("(o p) d -> p o d", p=P)
    outT_r = outT.ap().rearrange("(o p) t -> p o t", p=P)

    with tile.TileContext(nc) as tc:
        with (
            tc.tile_pool(name="weights", bufs=1) as wpool,
            tc.tile_pool(name="xin", bufs=1) as xpool,
            tc.tile_pool(name="hact", bufs=1) as hpool,
            tc.tile_pool(name="tmp", bufs=4) as tpool,
            tc.tile_pool(name="outp", bufs=4) as opool,
            tc.tile_pool(name="ps", bufs=1, space="PSUM") as ps_pool,
        ):
            wfc_sb = wpool.tile([P, KD, HID_S], F16)
            wproj_sb = wpool.tile([P, JC, DIM], F16)

            ps_tags = ["psA", "psB", "psC", "psD"][:n_tags]
            grp = [0]  # rotating group counter across warmup/mm1/mm2/reps

            def next_ps():
                tag = ps_tags[grp[0] % n_tags]
                grp[0] += 1
                return ps_pool.tile([P, t_half], F32, tag=tag, name=tag)

            # PE prewarm bridges the input-DMA wait and warms the HAM gate.
            warm_sb = wpool.tile([P, T_CHUNK], F16)
            nc.gpsimd.memset(warm_sb[:], 0.0)
            for i in range(24):
                ps_w = next_ps() if i % 8 == 0 else ps_w
                nc.tensor.matmul(ps_w[:, :T_CHUNK], lhsT=warm_sb[:, :P],
                                 rhs=warm_sb[:], start=True, stop=True)

            # wfc's first slice and x's first chunk gate the first matmuls.
            x_sb = xpool.tile([P, KD, T_S], F16)
            H_SPLIT = 256
            nc.sync.dma_start(wfc_sb[:, :, 0:H_SPLIT], wfcT_r[:, :, 0:H_SPLIT])
            nc.sync.dma_start(x_sb[:, :, 0:T_HALF], xT_r[:, :, 0:T_HALF])
            nc.sync.dma_start(x_sb[:, :, T_HALF:], xT_r[:, :, T_HALF:])
            for js in range(1, HID_S // H_SPLIT):
                sl = slice(js * H_SPLIT, (js + 1) * H_SPLIT)
                nc.sync.dma_start(wfc_sb[:, :, sl], wfcT_r[:, :, sl])
            for js in range(4):
                sl = slice(js * (JC // 4), (js + 1) * (JC // 4))
                nc.sync.dma_start(wproj_sb[:, sl, :], wprojT_r[:, sl, :])

            def mm1_phase(h_sb):
                for j in range(JC):
                    for th in range(th_groups):
                        t0 = th * t_half
                        ps = next_ps()
                        for k in range(KD):
                            for t in range(0, nth, tsub):
                                mm = nc.tensor.matmul(
                                    ps[:, t * T_CHUNK:t * T_CHUNK + mm_n],
                                    lhsT=wfc_sb[:, k, j * P:(j + 1) * P],
                                    rhs=x_sb[:, k,
                                             t0 + t * T_CHUNK:t0 + t * T_CHUNK + mm_n],
                                    start=(k == 0),
                                    stop=(k == KD - 1),
                                )
                                if t != 0:
                                    mm.ins.ldweights = False
                        if act_mode == "stt":
                            # relu(ps)^2 = (ps max 0) * ps in one DVE pass
                            nc.vector.scalar_tensor_tensor(
                                out=h_sb[:, j, t0:t0 + t_half],
                                in0=ps[:], scalar=0.0, in1=ps[:],
                                op0=mybir.AluOpType.max,
                                op1=mybir.AluOpType.mult,
                            )
                        else:
                            relu_t = tpool.tile([P, t_half], F16, tag="relu")
                            if relu_engine == "act":
                                nc.scalar.activation(
                                    relu_t[:], ps[:],
                                    mybir.ActivationFunctionType.Relu,
                                )
                            else:
                                nc.vector.tensor_scalar_max(relu_t[:], ps[:], 0.0)
                            nc.vector.tensor_mul(
                                out=h_sb[:, j, t0:t0 + t_half],
                                in0=relu_t[:], in1=relu_t[:],
                            )

            def mm2_phase(h_sb):
                for dc in range(DC):
                    # dc=0 reuses the tag drained n_tags groups back.
                    for th in range(th_groups):
                        t0 = th * t_half
                        po = next_ps()
                        for j in range(JC):
                            for t in range(0, nth, tsub):
                                mm = nc.tensor.matmul(
                                    po[:, t * T_CHUNK:t * T_CHUNK + mm_n],
                                    lhsT=wproj_sb[:, j, dc * P:(dc + 1) * P],
                                    rhs=h_sb[:, j,
                                             t0 + t * T_CHUNK:t0 + t * T_CHUNK + mm_n],
                                    start=(j == 0),
                                    stop=(j == JC - 1),
                                )
                                if t != 0:
                                    mm.ins.ldweights = False
                        o_sb = opool.tile([P, t_half], F16, tag="o")
                        nc.scalar.activation(
                            o_sb[:], po[:],
                            mybir.ActivationFunctionType.Copy,
                        )
                        if not skip_dma:
                            nc.sync.dma_start(outT_r[:, dc, t0:t0 + t_half],
                                              o_sb[:])

            h_shared = [None]

            def body(_iv=None):
                if parts == "mm2":
                    h_sb = h_shared[0]
                else:
                    h_sb = hpool.tile([P, JC, T_S], ACT, tag="h")
                if parts in ("both", "mm1"):
                    mm1_phase(h_sb)
                if parts in ("both", "mm2"):
                    mm2_phase(h_sb)

            if parts == "mm2":
                # produce real activations once, outside the timed loop
                h_init = hpool.tile([P, JC, T_S], ACT, tag="h", name="h_init")
                h_shared[0] = h_init
                mm1_phase(h_init)

            body()
            if reps > 1:
                n = reps - 1
                U = unroll if unroll is not None else UNROLL
                while n % U != 0:
                    U -= 1
                if reps <= 4:
                    for _ in range(n):
                        body()
                else:
                    with tc.For_i(0, n // U, 1,
                                  staggered_reset=staggered) as iv:
                        for _ in range(U):
                            body(iv)

    nc.compile()
    return nc


_NC_CACHE = {}

# Winning configuration (set from probe A/Bs; defaults = legacy baseline).
CFG = dict(act_dt="f16", act_mode="maxmul", nth=2, unroll=None)


def _get_nc(reps: int = 1):
    key = (reps, tuple(sorted(CFG.items())))
    if key not in _NC_CACHE:
        _NC_CACHE[key] = build_nc(reps, **CFG)
    return _NC_CACHE[key]


def _trunc16(a, m=6):
    """Round fp16 array to m mantissa bits (round-half-up, carry-safe).

    The PE multiplies at FP22 internally; zeroed low mantissa bits toggle
    fewer multiplier lines, which raises the power-throttled sustained
    clock (~12us/body measured vs full mantissa). Quantization error at
    m=6 contributes ~9e-3 scale-relative absmax, well under the 2e-2 gate.
    """
    u = a.view(np.uint16).astype(np.uint32)
    shift = 10 - m
    half = 1 << (shift - 1)
    mask = (~((1 << shift) - 1)) & 0xFFFF
    return ((u + half) & mask).astype(np.uint16).view(np.float16)


def make_in_maps(x, W_fc, W_proj):
    xT = _trunc16(np.ascontiguousarray(x.T.astype(np.float16)), 5)  # [DIM, T]
    wfcT16 = {}
    wprojT16 = {}
    for hid in range(HID_WAYS):
        hsl = slice(hid * HID_S, (hid + 1) * HID_S)
        wfcT16[hid] = _trunc16(
            np.ascontiguousarray(W_fc[hsl, :].T.astype(np.float16)), 6)
        wprojT16[hid] = _trunc16(
            np.ascontiguousarray(W_proj[:, hsl].T.astype(np.float16)), 6)
    in_maps = []
    for c in range(N_CORES):
        tok, hid = c // HID_WAYS, c % HID_WAYS
        in_maps.append({
            "xT": np.ascontiguousarray(xT[:, tok * T_S:(tok + 1) * T_S]),
            "wfcT": wfcT16[hid],
            "wprojT": wprojT16[hid],
        })
    return in_maps


def assemble_out(results):
    out = np.empty((T, DIM), dtype=np.float32)
    for tok in range(TOK_WAYS):
        acc = results[tok * HID_WAYS]["outT"].astype(np.float32)
        for hid in range(1, HID_WAYS):
            acc += results[tok * HID_WAYS + hid]["outT"].astype(np.float32)
        out[tok * T_S:(tok + 1) * T_S] = acc.T
    return out


def kernel(x, W_fc, W_proj):
    assert x.shape == (T, DIM) and W_fc.shape == (HID, DIM) and W_proj.shape == (DIM, HID)
    nc = _get_nc(reps=1)
    in_maps = make_in_maps(
        np.asarray(x, np.float32),
        np.asarray(W_fc, np.float32),
        np.asarray(W_proj, np.float32),
    )
    res = bass_utils.run_bass_kernel_spmd(nc, in_maps, core_ids=list(range(N_CORES)))
    return assemble_out(res.results)



# revision 21
# speedup vs baseline: 1.0018x; 1.0018x over previous
"""Trainium2 Bass kernel for ExpertMLP: out = relu(x @ W_fc.T)^2 @ W_proj.T.

Sharding: 4-way tokens x 2-way hidden across 8 NeuronCores.
Each core computes a partial out^T[:, t_shard] contracted over its hidden
half; the host sums the two hidden halves (fp16 partials, upcast to fp32)
and transposes while unsharding.

Per-core kernel (T_S=2048 tokens, HID_S=2048 hidden, DIM=1024), fp16
matmul operands with fp32 PSUM accumulation:
  mm1: h^T[j, t] = W_fcT-chunks.T @ xT-chunks     (PSUM accum over d)
  act: relu^2 (DVE max(ps,0) -> fp16, DVE square)
  mm2: out^T[d, t] = W_projT-chunks.T @ h^T-chunks (PSUM accum over j)

Measured design notes (tight reps-delta benches, 8-core SPMD):
- Sustained matmul rate is data-dependent (power throttling): all-zero
  fp16 operands stream at ~216 ns/MM (2.4 GHz), dense-random fp16 at
  ~310 ns/MM. (bf16 measured faster on a matmul-only probe but slower
  in the full kernel, so operands stay fp16.)
- Inputs are rounded host-side (fp16 kept): x to 5 mantissa bits,
  weights to 6. The PE multiplies at FP22, so zeroed low mantissa bits
  toggle fewer partial products, raising the throttled clock (~15us/body
  total). The moving operand's bits matter most (they stream through the
  array every cycle; stationary weight bits are static gates — weights
  at m=5 measured no further gain). Quantization cost: rel err 9.9e-3
  vs 6.8e-4 untruncated (gate 2e-2).
- PSUM accumulation groups are [P, 1024] 2-bank tiles on a 4-deep tag
  rotation, so a bank pair is reused 3 groups (~10us) after its drain.
- mm2 evictions run on ScalarE (Copy activation, fp32->fp16); mm1 drains
  on DVE. Splitting PSUM readers across engines measured fastest.
- For_i carries an all-engine barrier per iteration; unrolling 2 bodies
  per iteration amortizes it (~4us/body).
- Weights and x stay SBUF-resident; only x (in, once) and out^T (out,
  per body) touch HBM. Output is fp16 partials (halves the out DMA).

Power-wall characterization (same-session reps-delta A/Bs, 2026-08-10):
- Structural floor: the SAME NEFF with x zeroed runs 221.5us =
  1024 MM x 216 ns (the warm 2.4 GHz rate) -- zero idle/stall slack.
  All of the ~47 ns/MM real-data overhead is power throttling.
- Phase split: mm1-only 265 ns/MM, mm2-only 266 ns/MM -- both phases
  equally throttled even though mm2 streams full-mantissa fp16 act.
- The operand-mantissa knob is saturated at m=5/6: truncating inputs
  further (x to m=2 AND w to m=4, data-only change on the same NEFF)
  measures 269.6us vs base 268.9us. Streaming act as fp8 E3M4
  (mixed-dtype matmul, rel err 1.65e-2) measures 267.9us: noise.
  Halving LDWEIGHTS count (nth=4, 2048-token PSUM groups) is also
  noise (271.2us). The residual throttle is pinned by value-entropy
  energy in the accumulate datapath (partial-sum ripple + PSUM RMW),
  which no operand format reaches; only all-zero streams drop it.
- scalar_tensor_tensor relu^2 fusion from PSUM is illegal: walrus
  rejects two non-scalar PSUM reads per instruction (NCC_IBVF027).
- Session-to-session drift of the throttled rate is ~5% (268-282us
  for identical NEFFs+data); A/B only within one process.
- Closed in round 5: N=1024 fp16 moving operand is ISA-illegal
  (NCC_IXCG864; one PSUM bank caps N at 512 fp32 outputs, and the
  cost model confirms 1 col/cycle for all 16-bit dtypes -- the
  "131 ns/MM @ N=512" doc line does not apply). Removing the out-DMA
  entirely is worth <=0.5% (267.5 vs 268.1/269.2 same-session), so
  fp8 output partials are not worth the accuracy. fp8 DoubleRow is
  unreachable: its pair interleave needs two j-rows per SBUF
  partition, a cross-partition transpose DVE/ACT/DMA cannot do
  efficiently, on top of failing the 2e-2 gate (e4m3 act = 3.2e-2).
- Round 6: relu on ACT instead of DVE is 1.2% slower (271.5 vs
  268.4/268.7): DVE-relu confirmed.
- Round 7 -- the throttle mechanism, mapped. Zeroing 75% of x token
  COLUMNS (data-only; zeroed tokens zero their h/act columns too)
  runs at the full 2.4 GHz floor (220.8us); zeroing 50% stays fully
  throttled (272.3us, slightly worse than dense: alternating
  zero/dense columns maximize bus toggling). So the clock is a STEP:
  a 2.4 GHz release threshold at ~30% of dense array energy, with
  2.0 GHz as an overshoot-tolerant floor state. Dense data cannot
  cross a ~70% energy gap (formats reach ~1%), and duty-cycling
  cannot beat the floor state (throughput <= theta x 2.4 < 2.0
  continuous). 268us is therefore the optimum for dense evaluation.
  Same round: bf16 operands are speed-parity (268.4 vs 269.3/269.7,
  fp22-upconvert argument confirmed; session-1's "bf16 faster
  matmul-only probe" was a window artifact) and accuracy-inadmissible
  at m=5/6 (HW rel 1.871e-2 == sim, vs 2e-2 gate): fp16 stays.
"""

import numpy as np

import concourse.mybir as mybir
import concourse.tile as tile
from concourse import bacc
from concourse import bass_utils

T, DIM, HID = 8192, 1024, 4096
N_CORES = 8
TOK_WAYS, HID_WAYS = 4, 2
T_S = T // TOK_WAYS        # 2048 tokens per core
HID_S = HID // HID_WAYS    # 2048 hidden units per core
P = 128
F32 = mybir.dt.float32
F16 = mybir.dt.float16

T_CHUNK = 512              # free dim per matmul (one PSUM bank)
NTH = 2                    # t-chunks per accumulation group
T_HALF = T_CHUNK * NTH     # 1024 tokens per group

KD = DIM // P              # 8 contraction chunks for mm1
JC = HID_S // P            # 16 j-chunks (also mm2 contraction chunks)
DC = DIM // P              # 8 output-dim chunks for mm2

UNROLL = 2                 # bodies per For_i iteration


def build_nc(reps: int = 1, relu_engine: str = "dve", staggered: bool = True,
             unroll: int | None = None, act_dt: str = "f16",
             act_mode: str = "maxmul", nth: int = NTH, parts: str = "both",
             mm_n: int = T_CHUNK, skip_dma: bool = False, op_dt: str = "f16"):
    """staggered=True defers the For_i semaphore-reset barrier past the
    per-iteration pipeline drain (measured ~1us/body vs plain in a
    same-window A/B; never worse on either estimator).

    act_dt: "f16" or "f8e3" — storage dtype of the relu^2 activations
      (mm2's moving operand). f8e3 (E3M4) halves the SBUF->PE streaming
      bus width for mm2; mixed-dtype matmul (fp8 moving x fp16
      stationary) is legal on trn2 (both are upconverted to FP22).
    act_mode: "maxmul" (DVE max + DVE mul, 2 ops) or "stt" (single DVE
      scalar_tensor_tensor computing (ps max 0) * ps = relu(ps)^2).
    nth: 512-token chunks per PSUM accumulation group (2 or 4). nth=4
      halves LDWEIGHTS count (each weight serves 2048 tokens) at the
      cost of a 2-deep (vs 4-deep) PSUM tag rotation.
    """
    OP = F16 if op_dt == "f16" else mybir.dt.bfloat16
    ACT = OP if act_dt == "f16" else mybir.dt.float8e3
    n_tags = 8 // nth          # PSUM: group = nth banks
    t_half = T_CHUNK * nth     # tokens per accumulation group
    th_groups = T_S // t_half
    assert mm_n % T_CHUNK == 0 and t_half % mm_n == 0
    tsub = mm_n // T_CHUNK     # 512-chunks per matmul instruction
    nc = bacc.Bacc("TRN2", target_bir_lowering=False, debug=False)
    xT = nc.dram_tensor("xT", [DIM, T_S], OP, kind="ExternalInput")
    wfcT = nc.dram_tensor("wfcT", [DIM, HID_S], OP, kind="ExternalInput")
    wprojT = nc.dram_tensor("wprojT", [HID_S, DIM], OP, kind="ExternalInput")
    outT = nc.dram_tensor("outT", [DIM, T_S], OP, kind="ExternalOutput")

    xT_r = xT.ap().rearrange("(o p) t -> p o t", p=P)
    wfcT_r = wfcT.ap().rearrange("(o p) h -> p o h", p=P)
    wprojT_r = wprojT.ap().rearrange("(o p) d -> p o d", p=P)
    outT_r = outT.ap().rearrange("(o p) t -> p o t", p=P)

    with tile.TileContext(nc) as tc:
        with (
            tc.tile_pool(name="weights", bufs=1) as wpool,
            tc.tile_pool(name="xin", bufs=1) as xpool,
            tc.tile_pool(name="hact", bufs=1) as hpool,
            tc.tile_pool(name="tmp", bufs=4) as tpool,
            tc.tile_pool(name="outp", bufs=4) as opool,
            tc.tile_pool(name="ps", bufs=1, space="PSUM") as ps_pool,
        ):
            wfc_sb = wpool.tile([P, KD, HID_S], OP)
            wproj_sb = wpool.tile([P, JC, DIM], OP)

            ps_tags = ["psA", "psB", "psC", "psD"][:n_tags]
            grp = [0]  # rotating group counter across warmup/mm1/mm2/reps

            def next_ps():
                tag = ps_tags[grp[0] % n_tags]
                grp[0] += 1
                return ps_pool.tile([P, t_half], F32, tag=tag, name=tag)

            # PE prewarm bridges the input-DMA wait and warms the HAM gate.
            warm_sb = wpool.tile([P, T_CHUNK], OP)
            nc.gpsimd.memset(warm_sb[:], 0.0)
            for i in range(24):
                ps_w = next_ps() if i % 8 == 0 else ps_w
                nc.tensor.matmul(ps_w[:, :T_CHUNK], lhsT=warm_sb[:, :P],
                                 rhs=warm_sb[:], start=True, stop=True)

            # wfc's first slice and x's first chunk gate the first matmuls.
            x_sb = xpool.tile([P, KD, T_S], OP)
            H_SPLIT = 256
            nc.sync.dma_start(wfc_sb[:, :, 0:H_SPLIT], wfcT_r[:, :, 0:H_SPLIT])
            nc.sync.dma_start(x_sb[:, :, 0:T_HALF], xT_r[:, :, 0:T_HALF])
            nc.sync.dma_start(x_sb[:, :, T_HALF:], xT_r[:, :, T_HALF:])
            for js in range(1, HID_S // H_SPLIT):
                sl = slice(js * H_SPLIT, (js + 1) * H_SPLIT)
                nc.sync.dma_start(wfc_sb[:, :, sl], wfcT_r[:, :, sl])
            for js in range(4):
                sl = slice(js * (JC // 4), (js + 1) * (JC // 4))
                nc.sync.dma_start(wproj_sb[:, sl, :], wprojT_r[:, sl, :])

            def mm1_phase(h_sb):
                for j in range(JC):
                    for th in range(th_groups):
                        t0 = th * t_half
                        ps = next_ps()
                        for k in range(KD):
                            for t in range(0, nth, tsub):
                                mm = nc.tensor.matmul(
                                    ps[:, t * T_CHUNK:t * T_CHUNK + mm_n],
                                    lhsT=wfc_sb[:, k, j * P:(j + 1) * P],
                                    rhs=x_sb[:, k,
                                             t0 + t * T_CHUNK:t0 + t * T_CHUNK + mm_n],
                                    start=(k == 0),
                                    stop=(k == KD - 1),
                                )
                                if t != 0:
                                    mm.ins.ldweights = False
                        if act_mode == "stt":
                            # relu(ps)^2 = (ps max 0) * ps in one DVE pass
                            nc.vector.scalar_tensor_tensor(
                                out=h_sb[:, j, t0:t0 + t_half],
                                in0=ps[:], scalar=0.0, in1=ps[:],
                                op0=mybir.AluOpType.max,
                                op1=mybir.AluOpType.mult,
                            )
                        else:
                            relu_t = tpool.tile([P, t_half], OP, tag="relu")
                            if relu_engine == "act":
                                nc.scalar.activation(
                                    relu_t[:], ps[:],
                                    mybir.ActivationFunctionType.Relu,
                                )
                            else:
                                nc.vector.tensor_scalar_max(relu_t[:], ps[:], 0.0)
                            nc.vector.tensor_mul(
                                out=h_sb[:, j, t0:t0 + t_half],
                                in0=relu_t[:], in1=relu_t[:],
                            )

            def mm2_phase(h_sb):
                for dc in range(DC):
                    # dc=0 reuses the tag drained n_tags groups back.
                    for th in range(th_groups):
                        t0 = th * t_half
                        po = next_ps()
                        for j in range(JC):
                            for t in range(0, nth, tsub):
                                mm = nc.tensor.matmul(
                                    po[:, t * T_CHUNK:t * T_CHUNK + mm_n],
                                    lhsT=wproj_sb[:, j, dc * P:(dc + 1) * P],
                                    rhs=h_sb[:, j,
                                             t0 + t * T_CHUNK:t0 + t * T_CHUNK + mm_n],
                                    start=(j == 0),
                                    stop=(j == JC - 1),
                                )
                                if t != 0:
                                    mm.ins.ldweights = False
                        o_sb = opool.tile([P, t_half], OP, tag="o")
                        nc.scalar.activation(
                            o_sb[:], po[:],
                            mybir.ActivationFunctionType.Copy,
                        )
                        if not skip_dma:
                            nc.sync.dma_start(outT_r[:, dc, t0:t0 + t_half],
                                              o_sb[:])

            h_shared = [None]

            def body(_iv=None):
                if parts == "mm2":
                    h_sb = h_shared[0]
                else:
                    h_sb = hpool.tile([P, JC, T_S], ACT, tag="h")
                if parts in ("both", "mm1"):
                    mm1_phase(h_sb)
                if parts in ("both", "mm2"):
                    mm2_phase(h_sb)

            if parts == "mm2":
                # produce real activations once, outside the timed loop
                h_init = hpool.tile([P, JC, T_S], ACT, tag="h", name="h_init")
                h_shared[0] = h_init
                mm1_phase(h_init)

            body()
            if reps > 1:
                n = reps - 1
                U = unroll if unroll is not None else UNROLL
                while n % U != 0:
                    U -= 1
                if reps <= 4:
                    for _ in range(n):
                        body()
                else:
                    with tc.For_i(0, n // U, 1,
                                  staggered_reset=staggered) as iv:
                        for _ in range(U):
                            body(iv)

    nc.compile()
    return nc


_NC_CACHE = {}

# Winning configuration (set from probe A/Bs; defaults = legacy baseline).
CFG = dict(act_dt="f16", act_mode="maxmul", nth=2, unroll=None)


def _get_nc(reps: int = 1):
    key = (reps, tuple(sorted(CFG.items())))
    if key not in _NC_CACHE:
        _NC_CACHE[key] = build_nc(reps, **CFG)
    return _NC_CACHE[key]


def _trunc16(a, m=6):
    """Round fp16 array to m mantissa bits (round-half-up, carry-safe).

    The PE multiplies at FP22 internally; zeroed low mantissa bits toggle
    fewer multiplier lines, which raises the power-throttled sustained
    clock (~12us/body measured vs full mantissa). Quantization error at
    m=6 contributes ~9e-3 scale-relative absmax, well under the 2e-2 gate.
    """
    u = a.view(np.uint16).astype(np.uint32)
    shift = 10 - m
    half = 1 << (shift - 1)
    mask = (~((1 << shift) - 1)) & 0xFFFF
    return ((u + half) & mask).astype(np.uint16).view(np.float16)


def make_in_maps(x, W_fc, W_proj):
    xT = _trunc16(np.ascontiguousarray(x.T.astype(np.float16)), 5)  # [DIM, T]
    wfcT16 = {}
    wprojT16 = {}
    for hid in range(HID_WAYS):
        hsl = slice(hid * HID_S, (hid + 1) * HID_S)
        wfcT16[hid] = _trunc16(
            np.ascontiguousarray(W_fc[hsl, :].T.astype(np.float16)), 6)
        wprojT16[hid] = _trunc16(
            np.ascontiguousarray(W_proj[:, hsl].T.astype(np.float16)), 6)
    in_maps = []
    for c in range(N_CORES):
        tok, hid = c // HID_WAYS, c % HID_WAYS
        in_maps.append({
            "xT": np.ascontiguousarray(xT[:, tok * T_S:(tok + 1) * T_S]),
            "wfcT": wfcT16[hid],
            "wprojT": wprojT16[hid],
        })
    return in_maps


def assemble_out(results):
    out = np.empty((T, DIM), dtype=np.float32)
    for tok in range(TOK_WAYS):
        acc = results[tok * HID_WAYS]["outT"].astype(np.float32)
        for hid in range(1, HID_WAYS):
            acc += results[tok * HID_WAYS + hid]["outT"].astype(np.float32)
        out[tok * T_S:(tok + 1) * T_S] = acc.T
    return out


def kernel(x, W_fc, W_proj):
    assert x.shape == (T, DIM) and W_fc.shape == (HID, DIM) and W_proj.shape == (DIM, HID)
    nc = _get_nc(reps=1)
    in_maps = make_in_maps(
        np.asarray(x, np.float32),
        np.asarray(W_fc, np.float32),
        np.asarray(W_proj, np.float32),
    )
    res = bass_utils.run_bass_kernel_spmd(nc, in_maps, core_ids=list(range(N_CORES)))
    return assemble_out(res.results)

